# revision 3
# baseline (speedup 1.0000x reference)
"""BiLIF (bidirectional leaky-integrate-and-fire) node on 8 Trainium2 NeuronCores.

Problem: inputs [T=16, B=64, N=65536] f32.
  s1 = LIF-scan(x,          tau=4/3, v_th=0.75)   (hard reset to 0)
  s2 = LIF-scan(flip(x, 0), tau=4/3, v_th=1.25)
  out = (s1 + s2) / 2

Strategy (v2 — 2x-mode DVE):
  - Batch dim sharded across 8 cores (pure data parallel). Per core
    8*65536 positions = 128 partitions x 4096 cols, one full-width tile
    per time step (all DMA is contiguous 1 MB transfers).
  - Arithmetic runs in a x4096 integer domain: the host quantizes
    x_i = int16(rint(4096*x)) (abs err 9.2e-5 in h units — 4-12x better
    than fp16(x)), and the device recurrence per step is
        q  = 0.25 * (x_i + vq)               (fp32 internal)
        vq' = q < Cq ? q : V*                Cq = 0.25*4096*th/0.75
    with vq stored fp16 and V* a fixed fp16-exact sentinel (-0.00196)
    written on spike via the SELECT ALU op. The spike is recovered
    exactly as vq == V* (a plain 0-sentinel collides with the atoms of
    h == 0 — integer x plus integral fp16 carry — which cost ~8k false
    spikes; V* sits on a mid-mantissa value no carry can hit). The
    combine op is  out = (vq1 == V*) + (vq2 == V*)  in {0,1,2},
    and the host multiplies by 0.5. Measured: 1930 flipped spikes
    / 67M, rel err 1.17e-2 (gate: 2e-2), bit-identical to the numpy
    model of the device arithmetic.
  - Both custom DVE ops carry hand-authored 2x_1P micro-op programs
    (two packed 16-bit elements per cycle; the 8-deep ALU pipeline fits
    two 4-op chains) and are emitted with perf_max=1. All operands are
    16-bit dense, so the engine runs them at 2 elem/lane/cycle — the
    fp32 1x baseline was DVE-bound; this version runs at the joint
    DVE/HBM roofline (~76-88 us vs 155 us baseline).
  - DMA: 16 MB int16 in + 16 MB fp16 out per core, contiguous.  Both
    directions run concurrently at step t (fwd consumes x[t], bwd
    x[15-t]); each x tile is loaded once.
"""

import numpy as np

import concourse.bacc as bacc
import concourse.bass_isa as bass_isa
import concourse.mybir as mybir
import concourse.tile as tile
import concourse.dve_ops as dve_ops
from concourse.dve_ops import DveOp
from concourse.dve_spec import Spec, Src0, Src1, C0, C1, Zero, eq, _has_src1
from concourse.dve_uop import (
    DveOpSpec, UopConfig, InpSel, OutSel, OutPath,
    AluOp, AluInp, DelayInp, Trigger,
)
from concourse.ap import AP
from concourse.bass_primitives import MemorySpace
from concourse.bass import assert_partition_dims_match
from concourse import bass_utils

T, B, N = 16, 64, 65536
NCORES = 8
BS = B // NCORES
POS = BS * N
P = 128
FREE = POS // P          # 4096
CHUNK = 4096
NCHUNK = FREE // CHUNK
S = 4096.0               # host quantization scale
CQ1 = float(0.25 * S * 0.75 / 0.75)   # 1024.0   (q-domain threshold, dir 1)
CQ2 = float(0.25 * S * 1.25 / 0.75)   # 1706.666 (q-domain threshold, dir 2)
QQ = 0.25
VSTAR = float(np.float16(-0.0019551))  # fp16-exact spike sentinel
F16 = mybir.dt.float16
I16 = mybir.dt.int16
VER = "v3"


# --------------- custom DVE ops (hand-authored 2x_1P programs) -------------- #

def _step_uop_1x():
    """out = (q < C0) ? q : C2,  q = (Src0 + Src1) * C1."""
    u = UopConfig()
    u.enable_input(InpSel.SRC_0, 0)
    u.enable_input(InpSel.SRC_1, 1)
    u.enable_input(InpSel.CONST_0, 4)   # Cq = 0.25*th''  -> chain 3
    u.enable_input(InpSel.CONST_1, 5)   # 0.25            -> chain 4
    u.enable_input(InpSel.CONST_2, 6)   # V*              -> chain 5
    dp = u.datapath_config
    dp[0].enable_alu(AluOp.ADD, AluInp.PREV_ALU_OUT, AluInp.PREV_DELAY_0)
    dp[0].pass_through_delay(3, 4, 5)
    dp[1].enable_alu(AluOp.MULTIPLY, AluInp.PREV_ALU_OUT, AluInp.PREV_DELAY_4)
    dp[1].pass_through_delay(3, 5)
    dp[2].enable_alu(AluOp.IS_LT, AluInp.PREV_ALU_OUT, AluInp.PREV_DELAY_3)
    dp[2].enable_delay_from_src(DelayInp.PREV_ALU_OUT, 0)  # d0 <- q
    dp[2].pass_through_delay(5)
    dp[3].enable_alu(AluOp.SELECT, AluInp.PREV_DELAY_5, AluInp.PREV_DELAY_0)
    for b in range(4, 8):
        dp[b].pass_through_alu()
    u.enable_output(OutSel.ALU_OUT, OutPath.WR0_LO)
    u.require_inp0 = 1
    u.require_inp1 = 1
    u.trigger = (Trigger.SRC_TENSOR_DONE, Trigger.NONE, Trigger.NONE)
    return u


def _step_uop_2x():
    u = UopConfig()
    u.enable_input(InpSel.SRC_0, 0)      # x_lo  -> ALU position
    u.enable_input(InpSel.SRC_1, 1)      # vq_lo -> chain 0
    u.enable_input(InpSel.SRC_0_HI, 2)   # x_hi  -> chain 1
    u.enable_input(InpSel.SRC_1_HI, 3)   # vq_hi -> chain 2
    u.enable_input(InpSel.CONST_0, 4)    # Cq    -> chain 3
    u.enable_input(InpSel.CONST_1, 5)    # 0.25  -> chain 4
    u.enable_input(InpSel.CONST_2, 6)    # V*    -> chain 5
    dp = u.datapath_config
    # low chain: blocks 0-3
    dp[0].enable_alu(AluOp.ADD, AluInp.PREV_ALU_OUT, AluInp.PREV_DELAY_0)
    dp[0].pass_through_delay(1, 2, 3, 4, 5)
    dp[1].enable_alu(AluOp.MULTIPLY, AluInp.PREV_ALU_OUT, AluInp.PREV_DELAY_4)
    dp[1].pass_through_delay(1, 2, 3, 4, 5)
    dp[2].enable_alu(AluOp.IS_LT, AluInp.PREV_ALU_OUT, AluInp.PREV_DELAY_3)
    dp[2].enable_delay_from_src(DelayInp.PREV_ALU_OUT, 0)  # d0 <- q_lo
    dp[2].pass_through_delay(1, 2, 3, 4, 5)
    dp[3].enable_alu(AluOp.SELECT, AluInp.PREV_DELAY_5, AluInp.PREV_DELAY_0)
    dp[3].pass_through_delay(1, 2, 3, 4, 5)
    # high chain: blocks 4-7; d0 carries out_lo to the write stage
    dp[4].enable_alu(AluOp.ADD, AluInp.PREV_DELAY_1, AluInp.PREV_DELAY_2)
    dp[4].enable_delay_from_src(DelayInp.PREV_ALU_OUT, 0)  # d0 <- out_lo
    dp[4].pass_through_delay(3, 4, 5)
    dp[5].enable_alu(AluOp.MULTIPLY, AluInp.PREV_ALU_OUT, AluInp.PREV_DELAY_4)
    dp[5].pass_through_delay(0, 3, 5)
    dp[6].enable_alu(AluOp.IS_LT, AluInp.PREV_ALU_OUT, AluInp.PREV_DELAY_3)
    dp[6].enable_delay_from_src(DelayInp.PREV_ALU_OUT, 1)  # d1 <- q_hi
    dp[6].pass_through_delay(0, 5)
    dp[7].enable_alu(AluOp.SELECT, AluInp.PREV_DELAY_5, AluInp.PREV_DELAY_1)
    dp[7].pass_through_delay(0)
    u.enable_output(OutSel.DELAY_0, OutPath.WR0_LO)   # out_lo
    u.enable_output(OutSel.ALU_OUT, OutPath.WR0_HI)   # out_hi
    u.require_inp0 = 1
    u.require_inp1 = 1
    u.trigger = (Trigger.SRC_TENSOR_DONE, Trigger.NONE, Trigger.NONE)
    return u


def _eq0_uop_1x():
    u = UopConfig()
    u.enable_input(InpSel.SRC_0, 0)
    u.enable_input(InpSel.SRC_1, 1)
    u.enable_input(InpSel.CONST_0, 4)   # V* -> chain 3
    dp = u.datapath_config
    dp[0].enable_alu(AluOp.IS_EQ, AluInp.PREV_ALU_OUT, AluInp.PREV_DELAY_3)
    dp[0].pass_through_delay(0, 3)
    dp[1].enable_alu(AluOp.IS_EQ, AluInp.PREV_DELAY_0, AluInp.PREV_DELAY_3)
    dp[1].enable_delay_from_src(DelayInp.PREV_ALU_OUT, 0)
    dp[2].enable_alu(AluOp.ADD, AluInp.PREV_ALU_OUT, AluInp.PREV_DELAY_0)
    for b in range(3, 8):
        dp[b].pass_through_alu()
    u.enable_output(OutSel.ALU_OUT, OutPath.WR0_LO)
    u.require_inp0 = 1
    u.require_inp1 = 1
    u.trigger = (Trigger.SRC_TENSOR_DONE, Trigger.NONE, Trigger.NONE)
    return u


def _eq0_uop_2x():
    u = UopConfig()
    u.enable_input(InpSel.SRC_0, 0)
    u.enable_input(InpSel.SRC_1, 1)
    u.enable_input(InpSel.SRC_0_HI, 2)
    u.enable_input(InpSel.SRC_1_HI, 3)
    u.enable_input(InpSel.CONST_0, 4)   # V* -> chain 3
    dp = u.datapath_config
    dp[0].enable_alu(AluOp.IS_EQ, AluInp.PREV_ALU_OUT, AluInp.PREV_DELAY_3)
    dp[0].pass_through_delay(0, 1, 2, 3)
    dp[1].enable_alu(AluOp.IS_EQ, AluInp.PREV_DELAY_0, AluInp.PREV_DELAY_3)
    dp[1].enable_delay_from_src(DelayInp.PREV_ALU_OUT, 0)
    dp[1].pass_through_delay(1, 2, 3)
    dp[2].enable_alu(AluOp.ADD, AluInp.PREV_ALU_OUT, AluInp.PREV_DELAY_0)
    dp[2].pass_through_delay(1, 2, 3)
    dp[3].enable_alu(AluOp.IS_EQ, AluInp.PREV_DELAY_1, AluInp.PREV_DELAY_3)
    dp[3].enable_delay_from_src(DelayInp.PREV_ALU_OUT, 0)
    dp[3].pass_through_delay(2, 3)
    dp[4].enable_alu(AluOp.IS_EQ, AluInp.PREV_DELAY_2, AluInp.PREV_DELAY_3)
    dp[4].enable_delay_from_src(DelayInp.PREV_ALU_OUT, 1)
    dp[4].pass_through_delay(0)
    dp[5].enable_alu(AluOp.ADD, AluInp.PREV_ALU_OUT, AluInp.PREV_DELAY_1)
    dp[5].pass_through_delay(0)
    for b in range(6, 8):
        dp[b].pass_through_alu()
        dp[b].pass_through_delay(0)
    u.enable_output(OutSel.DELAY_0, OutPath.WR0_LO)
    u.enable_output(OutSel.ALU_OUT, OutPath.WR0_HI)
    u.require_inp0 = 1
    u.require_inp1 = 1
    u.trigger = (Trigger.SRC_TENSOR_DONE, Trigger.NONE, Trigger.NONE)
    return u


def _step_ref(in0, in1, s0, s1, imm2):
    h = in0.astype(np.float32) + in1.astype(np.float32)
    q = (h * np.float32(s1)).astype(np.float32)
    return np.where(q < np.float32(s0), q, np.float32(imm2))


def _eq0_ref(in0, in1, s0, s1, imm2):
    return ((in0 == np.float32(s0)).astype(np.float32)
            + (in1 == np.float32(s0)).astype(np.float32))


def _register_fast(name, spec, uop_1x, uop_2x):
    if name in dve_ops._SUB_OPCODE_FOR_NAME:
        for op in dve_ops.OPS:
            if op.name == name:
                return op
    row = dve_ops._CUSTOM_DVE_ROW_BASE + len(dve_ops.OPS)
    assert row < 0x20, "custom DVE opcode rows exhausted"
    for u in (uop_1x, uop_2x):
        u.validate(VER)
    compiled = DveOpSpec(name=name, opcode=row, uops=[uop_1x],
                         uops_2x=[uop_2x], perf_max=1,
                         rd1_en=_has_src1(spec))
    op = DveOp(name, spec, subdim=False, uops_sha={VER: compiled.sha(VER)})
    dve_ops.OPS.append(op)
    dve_ops._SUB_OPCODE_FOR_NAME[name] = row
    dve_ops.CUSTOM_DVE_SPECS[name] = spec
    dve_ops._COMPILE_CACHE[(name, VER)] = compiled
    return op


from concourse.dve_spec import C2, select  # noqa: E402

_q = (Src0 + Src1) * C1
BILIF_STEP3 = _register_fast(
    "BILIF_STEP3", Spec(body=select(_q < C0, _q, C2), reference=_step_ref),
    _step_uop_1x(), _step_uop_2x())
BILIF_EQV = _register_fast(
    "BILIF_EQV", Spec(body=eq(Src0, C0) + eq(Src1, C0), reference=_eq0_ref),
    _eq0_uop_1x(), _eq0_uop_2x())


def _emit(vec, op, *, out, in0, in1, s0=0.0, s1=0.0, imm2=0.0):
    """nc.vector._custom_dve with the instruction perf_max field set, so the
    engine may engage the hand-authored 2x_1P table slot."""
    nc_bass = vec.bass
    if op.name not in nc_bass.m.ant_custom_dve_ops:
        nc_bass.m.ant_custom_dve_ops = sorted(
            {*nc_bass.m.ant_custom_dve_ops, op.name})
    for ap in (out, in0, in1):
        assert ap.space in (MemorySpace.SBUF, MemorySpace.PSUM)
    assert_partition_dims_match(out, in0, in1)
    shape = bass_isa.CustomDveShape.TTSS
    isa_opcode = nc_bass.isa.Opcode[
        f"NEURON_ISA_TPB_OPCODE_CUSTOM_DVE_ANT_{shape.slot()}"].value

    def imm(v):
        return mybir.ImmediateValue(dtype=mybir.dt.float32, value=float(v))

    return vec.add_instruction(bass_isa.InstCustomDveAnt(
        name=nc_bass.get_next_instruction_name(),
        op_name=op.name, rd1_en=True, subdim=0, imm2=float(imm2), shape=shape,
        row=dve_ops.get_dve_sub_opcode(op.name), isa_opcode=isa_opcode,
        perf_max=1,
        ins=[vec.lower_ap(in0, for_isa=True, opt=True),
             vec.lower_ap(in1, for_isa=True, opt=True), imm(s0), imm(s1)],
        outs=[vec.lower_ap(out, for_isa=True, opt=True)]))


# ------------------------------- the kernel -------------------------------- #

_NC_CACHE = {}


def _build_nc(repeat: int = 1):
    key = repeat
    if key in _NC_CACHE:
        return _NC_CACHE[key]
    nc = bacc.Bacc("TRN2", target_bir_lowering=False, debug=False,
                   num_devices=NCORES)
    x_d = nc.dram_tensor("x", [T * P, FREE], I16, kind="ExternalInput").ap()
    o_d = nc.dram_tensor("o", [T * P, FREE], F16, kind="ExternalOutput").ap()

    with tile.TileContext(nc) as tc:
        with tc.tile_pool(name="xp", bufs=16) as xp, \
             tc.tile_pool(name="v1p", bufs=2) as v1p, \
             tc.tile_pool(name="v2p", bufs=2) as v2p, \
             tc.tile_pool(name="outp", bufs=3) as outp, \
             tc.tile_pool(name="zp", bufs=1) as zp:
            zt = zp.tile([P, CHUNK], F16, tag="z", name="z")
            nc.vector.memset(zt[:], 0.0)
            for rep in range(repeat):
                for k in range(NCHUNK):
                    c0 = k * CHUNK
                    xt = {}
                    for t in [v for s_ in range(T // 2) for v in (s_, T - 1 - s_)]:
                        xt[t] = xp.tile([P, CHUNK], I16, tag="x",
                                        name=f"x{rep}_{k}_{t}")
                        nc.sync.dma_start(
                            out=xt[t][:],
                            in_=x_d[t * P:(t + 1) * P, c0:c0 + CHUNK])
                    v1_prev, v2_prev = zt, zt
                    for t in range(T):
                        v1 = v1p.tile([P, CHUNK], F16, tag="v1", name="v1")
                        v2 = v2p.tile([P, CHUNK], F16, tag="v2", name="v2")
                        _emit(nc.vector, BILIF_STEP3, out=v1[:],
                              in0=xt[t][:], in1=v1_prev[:], s0=CQ1, s1=QQ,
                              imm2=VSTAR)
                        _emit(nc.vector, BILIF_STEP3, out=v2[:],
                              in0=xt[T - 1 - t][:], in1=v2_prev[:],
                              s0=CQ2, s1=QQ, imm2=VSTAR)
                        o = outp.tile([P, CHUNK], F16, tag="o", name="o")
                        _emit(nc.vector, BILIF_EQV, out=o[:], in0=v1[:],
                              in1=v2[:], s0=VSTAR)
                        nc.sync.dma_start(
                            out=o_d[t * P:(t + 1) * P, c0:c0 + CHUNK],
                            in_=o[:])
                        v1_prev, v2_prev = v1, v2

    nc.compile()
    _NC_CACHE[key] = nc
    return nc


def _quantize(inputs: np.ndarray) -> np.ndarray:
    return np.clip(np.rint(inputs * np.float32(S)), -32767, 32767).astype(np.int16)


def _run(inputs: np.ndarray, repeat: int = 1, **kwargs):
    nc = _build_nc(repeat)
    xi = _quantize(inputs)
    in_maps = []
    for c in range(NCORES):
        shard = np.ascontiguousarray(
            xi[:, c * BS:(c + 1) * BS, :]).reshape(T * P, FREE)
        in_maps.append({"x": shard})
    return bass_utils.run_bass_kernel_spmd(
        nc, in_maps, core_ids=list(range(NCORES)), **kwargs)


def kernel(inputs: np.ndarray, **kwargs) -> np.ndarray:
    inputs = np.asarray(inputs)
    assert inputs.shape == (T, B, N) and inputs.dtype == np.float32
    res = None
    err = None
    for _attempt in range(3):  # retry transient device faults
        try:
            res = _run(inputs, **kwargs)
            break
        except Exception as e:  # noqa: BLE001
            err = e
    if res is None:
        raise err
    out = np.empty((T, B, N), np.float32)
    for c in range(NCORES):
        out[:, c * BS:(c + 1) * BS, :] = (
            res.results[c]["o"].astype(np.float32) * np.float32(0.5)
        ).reshape(T, BS, N)
    return out


# revision 4
# speedup vs baseline: 1.1474x; 1.1474x over previous
"""BiLIF (bidirectional leaky-integrate-and-fire) node on 8 Trainium2 NeuronCores.

Problem: inputs [T=16, B=64, N=65536] f32.
  s1 = LIF-scan(x,          tau=4/3, v_th=0.75)   (hard reset to 0)
  s2 = LIF-scan(flip(x, 0), tau=4/3, v_th=1.25)
  out = (s1 + s2) / 2

Strategy (v2 — 2x-mode DVE):
  - Batch dim sharded across 8 cores (pure data parallel). Per core
    8*65536 positions = 128 partitions x 4096 cols, one full-width tile
    per time step (all DMA is contiguous 1 MB transfers).
  - Arithmetic runs in a x4096 integer domain: the host quantizes
    x_i = int16(rint(4096*x)) (abs err 9.2e-5 in h units — 4-12x better
    than fp16(x)), and the device recurrence per step is
        q  = 0.25 * (x_i + vq)               (fp32 internal)
        vq' = q < Cq ? q : V*                Cq = 0.25*4096*th/0.75
    with vq stored fp16 and V* a fixed fp16-exact sentinel (-0.00196)
    written on spike via the SELECT ALU op. The spike is recovered
    exactly as vq == V* (a plain 0-sentinel collides with the atoms of
    h == 0 — integer x plus integral fp16 carry — which cost ~8k false
    spikes; V* sits on a mid-mantissa value no carry can hit). The
    combine op is  out = (vq1 == V*) + (vq2 == V*)  in {0,1,2},
    and the host multiplies by 0.5. Measured: 1930 flipped spikes
    / 67M, rel err 1.17e-2 (gate: 2e-2), bit-identical to the numpy
    model of the device arithmetic.
  - Both custom DVE ops carry hand-authored 2x_1P micro-op programs
    (two packed 16-bit elements per cycle; the 8-deep ALU pipeline fits
    two 4-op chains) and are emitted with perf_max=1. All operands are
    16-bit dense, so the engine runs them at 2 elem/lane/cycle — the
    fp32 1x baseline was DVE-bound; this version runs at the joint
    DVE/HBM roofline (~76-88 us vs 155 us baseline).
  - DMA: 16 MB int16 in + 16 MB fp16 out per core, contiguous.  Both
    directions run concurrently at step t (fwd consumes x[t], bwd
    x[15-t]); each x tile is loaded once.
"""

import numpy as np

import concourse.bacc as bacc
import concourse.bass_isa as bass_isa
import concourse.mybir as mybir
import concourse.tile as tile
import concourse.dve_ops as dve_ops
from concourse.dve_ops import DveOp
from concourse.dve_spec import Spec, Src0, Src1, C0, C1, Zero, eq, _has_src1
from concourse.dve_uop import (
    DveOpSpec, UopConfig, InpSel, OutSel, OutPath,
    AluOp, AluInp, DelayInp, Trigger,
)
from concourse.ap import AP
from concourse.bass_primitives import MemorySpace
from concourse.bass import assert_partition_dims_match
from concourse import bass_utils

T, B, N = 16, 64, 65536
NCORES = 8
BS = B // NCORES
POS = BS * N
P = 128
FREE = POS // P          # 4096
CHUNK = 2048
NCHUNK = FREE // CHUNK
S = 4096.0               # host quantization scale
CQ1 = float(0.25 * S * 0.75 / 0.75)   # 1024.0   (q-domain threshold, dir 1)
CQ2 = float(0.25 * S * 1.25 / 0.75)   # 1706.666 (q-domain threshold, dir 2)
QQ = 0.25
VSTAR = float(np.float16(-0.0019551))  # fp16-exact spike sentinel
F16 = mybir.dt.float16
I16 = mybir.dt.int16
VER = "v3"


# --------------- custom DVE ops (hand-authored 2x_1P programs) -------------- #

def _step_uop_1x():
    """out = (q < C0) ? q : C2,  q = (Src0 + Src1) * C1."""
    u = UopConfig()
    u.enable_input(InpSel.SRC_0, 0)
    u.enable_input(InpSel.SRC_1, 1)
    u.enable_input(InpSel.CONST_0, 4)   # Cq = 0.25*th''  -> chain 3
    u.enable_input(InpSel.CONST_1, 5)   # 0.25            -> chain 4
    u.enable_input(InpSel.CONST_2, 6)   # V*              -> chain 5
    dp = u.datapath_config
    dp[0].enable_alu(AluOp.ADD, AluInp.PREV_ALU_OUT, AluInp.PREV_DELAY_0)
    dp[0].pass_through_delay(3, 4, 5)
    dp[1].enable_alu(AluOp.MULTIPLY, AluInp.PREV_ALU_OUT, AluInp.PREV_DELAY_4)
    dp[1].pass_through_delay(3, 5)
    dp[2].enable_alu(AluOp.IS_LT, AluInp.PREV_ALU_OUT, AluInp.PREV_DELAY_3)
    dp[2].enable_delay_from_src(DelayInp.PREV_ALU_OUT, 0)  # d0 <- q
    dp[2].pass_through_delay(5)
    dp[3].enable_alu(AluOp.SELECT, AluInp.PREV_DELAY_5, AluInp.PREV_DELAY_0)
    for b in range(4, 8):
        dp[b].pass_through_alu()
    u.enable_output(OutSel.ALU_OUT, OutPath.WR0_LO)
    u.require_inp0 = 1
    u.require_inp1 = 1
    u.trigger = (Trigger.SRC_TENSOR_DONE, Trigger.NONE, Trigger.NONE)
    return u


def _step_uop_2x():
    u = UopConfig()
    u.enable_input(InpSel.SRC_0, 0)      # x_lo  -> ALU position
    u.enable_input(InpSel.SRC_1, 1)      # vq_lo -> chain 0
    u.enable_input(InpSel.SRC_0_HI, 2)   # x_hi  -> chain 1
    u.enable_input(InpSel.SRC_1_HI, 3)   # vq_hi -> chain 2
    u.enable_input(InpSel.CONST_0, 4)    # Cq    -> chain 3
    u.enable_input(InpSel.CONST_1, 5)    # 0.25  -> chain 4
    u.enable_input(InpSel.CONST_2, 6)    # V*    -> chain 5
    dp = u.datapath_config
    # low chain: blocks 0-3
    dp[0].enable_alu(AluOp.ADD, AluInp.PREV_ALU_OUT, AluInp.PREV_DELAY_0)
    dp[0].pass_through_delay(1, 2, 3, 4, 5)
    dp[1].enable_alu(AluOp.MULTIPLY, AluInp.PREV_ALU_OUT, AluInp.PREV_DELAY_4)
    dp[1].pass_through_delay(1, 2, 3, 4, 5)
    dp[2].enable_alu(AluOp.IS_LT, AluInp.PREV_ALU_OUT, AluInp.PREV_DELAY_3)
    dp[2].enable_delay_from_src(DelayInp.PREV_ALU_OUT, 0)  # d0 <- q_lo
    dp[2].pass_through_delay(1, 2, 3, 4, 5)
    dp[3].enable_alu(AluOp.SELECT, AluInp.PREV_DELAY_5, AluInp.PREV_DELAY_0)
    dp[3].pass_through_delay(1, 2, 3, 4, 5)
    # high chain: blocks 4-7; d0 carries out_lo to the write stage
    dp[4].enable_alu(AluOp.ADD, AluInp.PREV_DELAY_1, AluInp.PREV_DELAY_2)
    dp[4].enable_delay_from_src(DelayInp.PREV_ALU_OUT, 0)  # d0 <- out_lo
    dp[4].pass_through_delay(3, 4, 5)
    dp[5].enable_alu(AluOp.MULTIPLY, AluInp.PREV_ALU_OUT, AluInp.PREV_DELAY_4)
    dp[5].pass_through_delay(0, 3, 5)
    dp[6].enable_alu(AluOp.IS_LT, AluInp.PREV_ALU_OUT, AluInp.PREV_DELAY_3)
    dp[6].enable_delay_from_src(DelayInp.PREV_ALU_OUT, 1)  # d1 <- q_hi
    dp[6].pass_through_delay(0, 5)
    dp[7].enable_alu(AluOp.SELECT, AluInp.PREV_DELAY_5, AluInp.PREV_DELAY_1)
    dp[7].pass_through_delay(0)
    u.enable_output(OutSel.DELAY_0, OutPath.WR0_LO)   # out_lo
    u.enable_output(OutSel.ALU_OUT, OutPath.WR0_HI)   # out_hi
    u.require_inp0 = 1
    u.require_inp1 = 1
    u.trigger = (Trigger.SRC_TENSOR_DONE, Trigger.NONE, Trigger.NONE)
    return u


def _eq0_uop_1x():
    u = UopConfig()
    u.enable_input(InpSel.SRC_0, 0)
    u.enable_input(InpSel.SRC_1, 1)
    u.enable_input(InpSel.CONST_0, 4)   # V* -> chain 3
    dp = u.datapath_config
    dp[0].enable_alu(AluOp.IS_EQ, AluInp.PREV_ALU_OUT, AluInp.PREV_DELAY_3)
    dp[0].pass_through_delay(0, 3)
    dp[1].enable_alu(AluOp.IS_EQ, AluInp.PREV_DELAY_0, AluInp.PREV_DELAY_3)
    dp[1].enable_delay_from_src(DelayInp.PREV_ALU_OUT, 0)
    dp[2].enable_alu(AluOp.ADD, AluInp.PREV_ALU_OUT, AluInp.PREV_DELAY_0)
    for b in range(3, 8):
        dp[b].pass_through_alu()
    u.enable_output(OutSel.ALU_OUT, OutPath.WR0_LO)
    u.require_inp0 = 1
    u.require_inp1 = 1
    u.trigger = (Trigger.SRC_TENSOR_DONE, Trigger.NONE, Trigger.NONE)
    return u


def _eq0_uop_2x():
    u = UopConfig()
    u.enable_input(InpSel.SRC_0, 0)
    u.enable_input(InpSel.SRC_1, 1)
    u.enable_input(InpSel.SRC_0_HI, 2)
    u.enable_input(InpSel.SRC_1_HI, 3)
    u.enable_input(InpSel.CONST_0, 4)   # V* -> chain 3
    dp = u.datapath_config
    dp[0].enable_alu(AluOp.IS_EQ, AluInp.PREV_ALU_OUT, AluInp.PREV_DELAY_3)
    dp[0].pass_through_delay(0, 1, 2, 3)
    dp[1].enable_alu(AluOp.IS_EQ, AluInp.PREV_DELAY_0, AluInp.PREV_DELAY_3)
    dp[1].enable_delay_from_src(DelayInp.PREV_ALU_OUT, 0)
    dp[1].pass_through_delay(1, 2, 3)
    dp[2].enable_alu(AluOp.ADD, AluInp.PREV_ALU_OUT, AluInp.PREV_DELAY_0)
    dp[2].pass_through_delay(1, 2, 3)
    dp[3].enable_alu(AluOp.IS_EQ, AluInp.PREV_DELAY_1, AluInp.PREV_DELAY_3)
    dp[3].enable_delay_from_src(DelayInp.PREV_ALU_OUT, 0)
    dp[3].pass_through_delay(2, 3)
    dp[4].enable_alu(AluOp.IS_EQ, AluInp.PREV_DELAY_2, AluInp.PREV_DELAY_3)
    dp[4].enable_delay_from_src(DelayInp.PREV_ALU_OUT, 1)
    dp[4].pass_through_delay(0)
    dp[5].enable_alu(AluOp.ADD, AluInp.PREV_ALU_OUT, AluInp.PREV_DELAY_1)
    dp[5].pass_through_delay(0)
    for b in range(6, 8):
        dp[b].pass_through_alu()
        dp[b].pass_through_delay(0)
    u.enable_output(OutSel.DELAY_0, OutPath.WR0_LO)
    u.enable_output(OutSel.ALU_OUT, OutPath.WR0_HI)
    u.require_inp0 = 1
    u.require_inp1 = 1
    u.trigger = (Trigger.SRC_TENSOR_DONE, Trigger.NONE, Trigger.NONE)
    return u


def _step_ref(in0, in1, s0, s1, imm2):
    h = in0.astype(np.float32) + in1.astype(np.float32)
    q = (h * np.float32(s1)).astype(np.float32)
    return np.where(q < np.float32(s0), q, np.float32(imm2))


def _eq0_ref(in0, in1, s0, s1, imm2):
    return ((in0 == np.float32(s0)).astype(np.float32)
            + (in1 == np.float32(s0)).astype(np.float32))


def _register_fast(name, spec, uop_1x, uop_2x):
    if name in dve_ops._SUB_OPCODE_FOR_NAME:
        for op in dve_ops.OPS:
            if op.name == name:
                return op
    row = dve_ops._CUSTOM_DVE_ROW_BASE + len(dve_ops.OPS)
    assert row < 0x20, "custom DVE opcode rows exhausted"
    for u in (uop_1x, uop_2x):
        u.validate(VER)
    compiled = DveOpSpec(name=name, opcode=row, uops=[uop_1x],
                         uops_2x=[uop_2x], perf_max=1,
                         rd1_en=_has_src1(spec))
    op = DveOp(name, spec, subdim=False, uops_sha={VER: compiled.sha(VER)})
    dve_ops.OPS.append(op)
    dve_ops._SUB_OPCODE_FOR_NAME[name] = row
    dve_ops.CUSTOM_DVE_SPECS[name] = spec
    dve_ops._COMPILE_CACHE[(name, VER)] = compiled
    return op


from concourse.dve_spec import C2, select  # noqa: E402

_q = (Src0 + Src1) * C1
BILIF_STEP3 = _register_fast(
    "BILIF_STEP3", Spec(body=select(_q < C0, _q, C2), reference=_step_ref),
    _step_uop_1x(), _step_uop_2x())
BILIF_EQV = _register_fast(
    "BILIF_EQV", Spec(body=eq(Src0, C0) + eq(Src1, C0), reference=_eq0_ref),
    _eq0_uop_1x(), _eq0_uop_2x())


def _emit(vec, op, *, out, in0, in1, s0=0.0, s1=0.0, imm2=0.0):
    """nc.vector._custom_dve with the instruction perf_max field set, so the
    engine may engage the hand-authored 2x_1P table slot."""
    nc_bass = vec.bass
    if op.name not in nc_bass.m.ant_custom_dve_ops:
        nc_bass.m.ant_custom_dve_ops = sorted(
            {*nc_bass.m.ant_custom_dve_ops, op.name})
    for ap in (out, in0, in1):
        assert ap.space in (MemorySpace.SBUF, MemorySpace.PSUM)
    assert_partition_dims_match(out, in0, in1)
    shape = bass_isa.CustomDveShape.TTSS
    isa_opcode = nc_bass.isa.Opcode[
        f"NEURON_ISA_TPB_OPCODE_CUSTOM_DVE_ANT_{shape.slot()}"].value

    def imm(v):
        return mybir.ImmediateValue(dtype=mybir.dt.float32, value=float(v))

    return vec.add_instruction(bass_isa.InstCustomDveAnt(
        name=nc_bass.get_next_instruction_name(),
        op_name=op.name, rd1_en=True, subdim=0, imm2=float(imm2), shape=shape,
        row=dve_ops.get_dve_sub_opcode(op.name), isa_opcode=isa_opcode,
        perf_max=1,
        ins=[vec.lower_ap(in0, for_isa=True, opt=True),
             vec.lower_ap(in1, for_isa=True, opt=True), imm(s0), imm(s1)],
        outs=[vec.lower_ap(out, for_isa=True, opt=True)]))


# ------------------------------- the kernel -------------------------------- #

_NC_CACHE = {}


def _build_nc(repeat: int = 1):
    key = repeat
    if key in _NC_CACHE:
        return _NC_CACHE[key]
    nc = bacc.Bacc("TRN2", target_bir_lowering=False, debug=False,
                   num_devices=NCORES)
    x_d = nc.dram_tensor("x", [T * NCHUNK * P, CHUNK], I16, kind="ExternalInput").ap()
    o_d = nc.dram_tensor("o", [T * NCHUNK * P, CHUNK], F16, kind="ExternalOutput").ap()

    with tile.TileContext(nc) as tc:
        with tc.tile_pool(name="xp", bufs=32) as xp, \
             tc.tile_pool(name="v1p", bufs=2) as v1p, \
             tc.tile_pool(name="v2p", bufs=2) as v2p, \
             tc.tile_pool(name="outp", bufs=3) as outp, \
             tc.tile_pool(name="zp", bufs=1) as zp:
            zt = zp.tile([P, CHUNK], F16, tag="z", name="z")
            nc.vector.memset(zt[:], 0.0)
            for rep in range(repeat):
                for k in range(NCHUNK):
                    c0 = k * CHUNK
                    xt = {}
                    for t in [v for s_ in range(T // 2) for v in (s_, T - 1 - s_)]:
                        xt[t] = xp.tile([P, CHUNK], I16, tag="x",
                                        name=f"x{rep}_{k}_{t}")
                        r0 = (t * NCHUNK + k) * P
                        nc.sync.dma_start(out=xt[t][:],
                                          in_=x_d[r0:r0 + P, :])
                    v1_prev, v2_prev = zt, zt
                    for t in range(T):
                        v1 = v1p.tile([P, CHUNK], F16, tag="v1", name="v1")
                        v2 = v2p.tile([P, CHUNK], F16, tag="v2", name="v2")
                        _emit(nc.vector, BILIF_STEP3, out=v1[:],
                              in0=xt[t][:], in1=v1_prev[:], s0=CQ1, s1=QQ,
                              imm2=VSTAR)
                        _emit(nc.vector, BILIF_STEP3, out=v2[:],
                              in0=xt[T - 1 - t][:], in1=v2_prev[:],
                              s0=CQ2, s1=QQ, imm2=VSTAR)
                        o = outp.tile([P, CHUNK], F16, tag="o", name="o")
                        _emit(nc.vector, BILIF_EQV, out=o[:], in0=v1[:],
                              in1=v2[:], s0=VSTAR)
                        ro = (t * NCHUNK + k) * P
                        nc.sync.dma_start(out=o_d[ro:ro + P, :], in_=o[:])
                        v1_prev, v2_prev = v1, v2

    nc.compile()
    _NC_CACHE[key] = nc
    return nc


def _quantize(inputs: np.ndarray) -> np.ndarray:
    return np.clip(np.rint(inputs * np.float32(S)), -32767, 32767).astype(np.int16)


def _prep_shard(xi: np.ndarray, c: int) -> np.ndarray:
    """Per-core device input: [T, NCHUNK, P, CHUNK] chunk-major rows so every
    [P, CHUNK] tile is one contiguous 512 KB DMA."""
    s = xi[:, c * BS:(c + 1) * BS, :].reshape(T, P, NCHUNK, CHUNK)
    return np.ascontiguousarray(s.transpose(0, 2, 1, 3)).reshape(
        T * NCHUNK * P, CHUNK)


def _run(inputs: np.ndarray, repeat: int = 1, **kwargs):
    nc = _build_nc(repeat)
    xi = _quantize(inputs)
    in_maps = [{"x": _prep_shard(xi, c)} for c in range(NCORES)]
    return bass_utils.run_bass_kernel_spmd(
        nc, in_maps, core_ids=list(range(NCORES)), **kwargs)


def kernel(inputs: np.ndarray, **kwargs) -> np.ndarray:
    inputs = np.asarray(inputs)
    assert inputs.shape == (T, B, N) and inputs.dtype == np.float32
    res = None
    err = None
    for _attempt in range(3):  # retry transient device faults
        try:
            res = _run(inputs, **kwargs)
            break
        except Exception as e:  # noqa: BLE001
            err = e
    if res is None:
        raise err
    out = np.empty((T, B, N), np.float32)
    for c in range(NCORES):
        r = res.results[c]["o"].reshape(T, NCHUNK, P, CHUNK).transpose(
            0, 2, 1, 3)
        out[:, c * BS:(c + 1) * BS, :] = (
            r.astype(np.float32) * np.float32(0.5)).reshape(T, BS, N)
    return out

OSHAPE = (T * NCHUNK * P, CHUNK)
